# revision 3
# baseline (speedup 1.0000x reference)
"""ChebConv(K=3) x3 GNN encoder on 8 trn2 NeuronCores (Bass/Tile).

Fused single-launch version: one Bass program runs all 3 layers with
on-device AllGather between phases.  Host stages per-core x shards +
edge metadata once; warm runs re-stage only x (51MB) and fetch the
output shards.

Strategy per core: 98 blocks x 128 dst slots; per 128-edge tile an
indirect-DMA row gather plus a one-hot selection matrix (iota==dloc)*w
built on DVE, reduced on the PE via P^T @ M with PSUM accumulation;
dense matmuls for the Chebyshev combine.
"""
import numpy as np
import jax
from jax.sharding import Mesh, PartitionSpec, NamedSharding
from jax.experimental.shard_map import shard_map

import concourse.bass as bass
import concourse.bacc as bacc
import concourse.mybir as mybir
from concourse.tile import TileContext
from concourse import bass2jax
from concourse.masks import make_identity

F32 = mybir.dt.float32
I32 = mybir.dt.int32
NCORES = 8


class Runner:
    def __init__(self, nc, n_cores=8):
        bass2jax.install_neuronx_cc_hook()
        self.nc = nc
        self.n_cores = n_cores
        partition_name = (
            nc.partition_id_tensor.name if nc.partition_id_tensor else None
        )
        in_names, out_names, out_avals = [], [], []
        for alloc in nc.m.functions[0].allocations:
            if not isinstance(alloc, mybir.MemoryLocationSet):
                continue
            name = alloc.memorylocations[0].name
            if alloc.kind == "ExternalInput":
                if name != partition_name:
                    in_names.append(name)
            elif alloc.kind == "ExternalOutput":
                out_names.append(name)
                out_avals.append(
                    jax.core.ShapedArray(
                        tuple(alloc.tensor_shape), mybir.dt.np(alloc.dtype)
                    )
                )
        self.in_names, self.out_names, self.out_avals = in_names, out_names, out_avals
        n_params = len(in_names)
        all_in_names = in_names + out_names + (
            [partition_name] if partition_name else []
        )

        def _body(*args):
            operands = list(args)
            if partition_name is not None:
                operands.append(bass2jax.partition_id_tensor())
            outs = bass2jax._bass_exec_p.bind(
                *operands,
                out_avals=tuple(out_avals),
                in_names=tuple(all_in_names),
                out_names=tuple(out_names),
                lowering_input_output_aliases=(),
                sim_require_finite=True,
                sim_require_nnan=True,
                nc=nc,
            )
            return tuple(outs)

        devices = jax.devices()[:n_cores]
        self.mesh = Mesh(np.asarray(devices), ("core",))
        self.sharding = NamedSharding(self.mesh, PartitionSpec("core"))
        nin = n_params + len(out_names)
        self.fn = jax.jit(
            shard_map(
                _body,
                mesh=self.mesh,
                in_specs=(PartitionSpec("core"),) * nin,
                out_specs=(PartitionSpec("core"),) * len(out_names),
                check_rep=False,
            ),
            keep_unused=True,
        )

    def put_one(self, name, arrs):
        """arrs: list of per-core arrays (len n_cores) or one replicated."""
        if not isinstance(arrs, (list, tuple)):
            arrs = [arrs] * self.n_cores
        cat = np.concatenate([np.asarray(a) for a in arrs], axis=0)
        return jax.device_put(cat, self.sharding)

    def zeros(self):
        return [
            jax.device_put(
                np.zeros((self.n_cores * a.shape[0], *a.shape[1:]), a.dtype),
                self.sharding,
            )
            for a in self.out_avals
        ]


class Cfg:
    def __init__(self, n_nodes, npc, blk=128):
        assert npc * NCORES == n_nodes
        self.N = n_nodes
        self.NPC = npc
        self.BLK = blk
        self.NB = -(-npc // blk)          # blocks per core
        self.SLOTS = self.NB * blk        # slots per core (>= npc)


# ---------------------------------------------------------------- host prep

def host_prep(cfg, edge_index):
    """Bin edges by (core, block) of dst, pad to T_fix 128-edge tiles.

    Gather-source ids are remapped to the per-core slot layout
    [NCORES*SLOTS, C].  Returns (metas, T_fix); metas[c] is
    [NB*128, 3*T_fix] f32; cols [0:T) src ids (int32 bitcast),
    [T:2T) dloc f32, [2T:3T) w f32.
    """
    N, NPC, BLK, NB = cfg.N, cfg.NPC, cfg.BLK, cfg.NB
    src = np.asarray(edge_index[0], dtype=np.int64)
    dst = np.asarray(edge_index[1], dtype=np.int64)
    mask = src != dst
    deg = np.bincount(src[mask], minlength=N).astype(np.float32)
    dinv = np.where(deg > 0, (1.0 / np.sqrt(np.maximum(deg, 1.0))).astype(np.float32), 0.0).astype(np.float32)
    w_all = (-dinv[src] * dinv[dst]).astype(np.float32)

    src = src[mask]
    dst = dst[mask]
    w = w_all[mask]

    order = np.argsort(dst, kind="stable")
    src, dst, w = src[order], dst[order], w[order]

    # slot-layout remap of gather sources
    src = (src // NPC) * cfg.SLOTS + (src % NPC)

    core = dst // NPC
    core_starts = np.searchsorted(core, np.arange(NCORES + 1))

    # per (core, block) counts to get global T_fix
    gb = (dst // BLK) if NPC % BLK == 0 else (core * NB + (dst - core * NPC) // BLK)
    cnt = np.bincount(gb, minlength=NCORES * NB)
    T_fix = int(-(-cnt.max() // 128))

    metas = []
    for c in range(NCORES):
        s, e = core_starts[c], core_starts[c + 1]
        cs, cd, cw = src[s:e], dst[s:e], w[s:e]
        b = (cd - c * NPC) // BLK
        dloc = (cd - c * NPC) % BLK
        bstart = np.searchsorted(b, np.arange(NB + 1))
        meta = np.zeros((NB, 128, 3 * T_fix), np.float32)
        idx_i32 = np.zeros((NB, 128, T_fix), np.int32)
        # vectorized scatter over the whole core
        n_in_blk = np.diff(bstart)
        pos = np.arange(len(cs)) - np.repeat(bstart[:-1], n_in_blk)
        t_i = pos // 128
        p_i = pos % 128
        bi = np.repeat(np.arange(NB), n_in_blk)
        idx_i32[bi, p_i, t_i] = cs.astype(np.int32)
        meta[bi, p_i, T_fix + t_i] = dloc.astype(np.float32)
        meta[bi, p_i, 2 * T_fix + t_i] = cw
        meta[:, :, 0:T_fix] = idx_i32.view(np.float32)
        metas.append(meta.reshape(NB * 128, 3 * T_fix))
    return metas, T_fix


def iota_host():
    return np.broadcast_to(np.arange(128, dtype=np.float32), (128, 128)).copy()


# ------------------------------------------------------------- program

def _p_build(nc, P_t, iota, m, T, t):
    """P_t[p, c] = (iota[c] == dloc[p]) * w[p]"""
    nc.vector.tensor_scalar(
        out=P_t[:], in0=iota[:],
        scalar1=m[:, T + t:T + t + 1],
        scalar2=m[:, 2 * T + t:2 * T + t + 1],
        op0=mybir.AluOpType.is_equal,
        op1=mybir.AluOpType.mult,
    )


def build_fused(cfg, T, dims=((128, 64, True), (64, 128, True), (128, 256, False)),
                unroll=2, no_gather=False, no_coll=False, reps=1):
    """One program: all 3 layers with on-device AllGather between phases.

    Inputs per core: xs (own x shard in slot layout [SLOTS, C0]), meta,
    iota, wk{0..2} [3*Cin, Cout], bias{0..2} [128, Cout].
    Output: h3 [SLOTS, Cout_last].

    no_gather/no_coll: timing-ablation variants (numerically wrong).
    reps: emit the whole model body N times (for dispatch-free timing).
    """
    NB, SLOTS = cfg.NB, cfg.SLOTS
    NG = NCORES * SLOTS
    C0 = dims[0][0]
    nc = bacc.Bacc("TRN2", target_bir_lowering=False, debug=False,
                   num_devices=NCORES)

    def gather_tile(gpool, v_ap, m, t, Cin):
        g = gpool.tile([128, Cin], F32, tag=f"g{t}")
        if no_gather:
            nc.sync.dma_start(out=g[:], in_=v_ap[t * 128:(t + 1) * 128, :])
        else:
            nc.gpsimd.indirect_dma_start(
                out=g[:], out_offset=None, in_=v_ap[:],
                in_offset=bass.IndirectOffsetOnAxis(
                    ap=m[:, t:t + 1].bitcast(I32), axis=0),
            )
        return g

    def allgather(src, dst):
        if no_coll:
            nc.sync.dma_start(out=dst[0:src.shape[0], :], in_=src[:])
        else:
            nc.gpsimd.collective_compute(
                "AllGather", mybir.AluOpType.bypass,
                replica_groups=[list(range(NCORES))],
                ins=[src[:]], outs=[dst[:]])

    xs_d = nc.declare_dram_parameter("xs", [SLOTS, C0], F32, isOutput=False)
    meta_d = nc.declare_dram_parameter("meta", [SLOTS, 3 * T], F32, isOutput=False)
    iota_d = nc.declare_dram_parameter("iota", [128, 128], F32, isOutput=False)
    wk_ds, bias_ds = [], []
    for li, (Cin, Cout, relu) in enumerate(dims):
        wk_ds.append(nc.declare_dram_parameter(f"wk{li}", [3 * Cin, Cout], F32, isOutput=False))
        bias_ds.append(nc.declare_dram_parameter(f"bias{li}", [128, Cout], F32, isOutput=False))
    out_d = nc.declare_dram_parameter("h3", [SLOTS, dims[-1][1]], F32, isOutput=True)

    # internal DRAM
    xb = nc.dram_tensor("xb", [SLOTS, C0], F32)
    xg = nc.dram_tensor("xg", [NG, C0], F32, addr_space="Shared")
    t1_s, t1_g, h_s, h_g = [], [], [], []
    for li, (Cin, Cout, relu) in enumerate(dims):
        t1_s.append(nc.dram_tensor(f"t1s{li}", [SLOTS, Cin], F32))
        t1_g.append(nc.dram_tensor(f"t1g{li}", [NG, Cin], F32, addr_space="Shared"))
        if li < len(dims) - 1:
            h_s.append(nc.dram_tensor(f"hs{li}", [SLOTS, Cout], F32))
            h_g.append(nc.dram_tensor(f"hg{li}", [NG, Cout], F32, addr_space="Shared"))
        else:
            h_s.append(None); h_g.append(None)

    with TileContext(nc) as tc:
        with tc.tile_pool(name="const", bufs=1) as cpool:
            iota = cpool.tile([128, 128], F32)
            nc.sync.dma_start(out=iota[:], in_=iota_d[:])
            ident = cpool.tile([128, 128], F32)
            make_identity(nc, ident[:])
            wks, biases = [], []
            for li, (Cin, Cout, relu) in enumerate(dims):
                row = []
                for k in range(3):
                    wt = cpool.tile([Cin, Cout], F32, tag=f"w{li}_{k}")
                    nc.sync.dma_start(out=wt[:], in_=wk_ds[li][k * Cin:(k + 1) * Cin, :])
                    row.append(wt)
                wks.append(row)
                bt = cpool.tile([128, Cout], F32, tag=f"b{li}")
                nc.sync.dma_start(out=bt[:], in_=bias_ds[li][:])
                biases.append(bt)

            def emit_prop(rep, li, Cin, v_prop):
                with (
                    tc.tile_pool(name=f"pl{rep}_{li}", bufs=2) as pool,
                    tc.tile_pool(name=f"gl{rep}_{li}", bufs=2) as gpool,
                    tc.tile_pool(name=f"ppl{rep}_{li}", bufs=2) as ppool,
                    tc.tile_pool(name=f"psl{rep}_{li}", bufs=2, space="PSUM") as psum,
                ):
                    def prop_body(i):
                        m = pool.tile([128, 3 * T], F32, tag="meta")
                        nc.sync.dma_start(out=m[:], in_=meta_d[bass.ds(i * 128, 128), :])
                        gs = [gather_tile(gpool, v_prop, m, t, Cin) for t in range(T)]
                        y_ps = psum.tile([128, Cin], F32, tag="yps")
                        for t in range(T):
                            P_t = ppool.tile([128, 128], F32, tag=f"P{t}")
                            _p_build(nc, P_t, iota, m, T, t)
                            nc.tensor.matmul(out=y_ps[:], lhsT=P_t[:], rhs=gs[t][:],
                                             start=(t == 0), stop=(t == T - 1))
                        y_sb = pool.tile([128, Cin], F32, tag="ysb")
                        nc.vector.tensor_copy(y_sb[:], y_ps[:])
                        nc.sync.dma_start(out=t1_s[li][bass.ds(i * 128, 128), :], in_=y_sb[:])

                    tc.For_i_unrolled(0, NB, 1, prop_body, max_unroll=unroll)

            def emit_combine(rep, li, Cin, Cout, relu, x0_src):
                with (
                    tc.tile_pool(name=f"cl{rep}_{li}", bufs=2) as pool,
                    tc.tile_pool(name=f"cgl{rep}_{li}", bufs=2) as gpool,
                    tc.tile_pool(name=f"cpl{rep}_{li}", bufs=2) as ppool,
                    tc.tile_pool(name=f"cs{rep}_{li}", bufs=2, space="PSUM") as psum,
                    tc.tile_pool(name=f"ct{rep}_{li}", bufs=2, space="PSUM") as psumt,
                ):
                    def comb_body(i):
                        m = pool.tile([128, 3 * T], F32, tag="meta")
                        nc.sync.dma_start(out=m[:], in_=meta_d[bass.ds(i * 128, 128), :])
                        gs = [gather_tile(gpool, t1_g[li], m, t, Cin) for t in range(T)]
                        s_ps = psum.tile([Cin, 128], F32, tag="sps")
                        for t in range(T):
                            P_t = ppool.tile([128, 128], F32, tag=f"P{t}")
                            _p_build(nc, P_t, iota, m, T, t)
                            nc.tensor.matmul(out=s_ps[:], lhsT=gs[t][:], rhs=P_t[:],
                                             start=(t == 0), stop=(t == T - 1))
                        # x0T via on-device transpose of the x0 block
                        xb_t = pool.tile([128, Cin], F32, tag="xb")
                        nc.sync.dma_start(out=xb_t[:], in_=x0_src[bass.ds(i * 128, 128), :])
                        xT_ps = psumt.tile([Cin, 128], F32, tag="xTps")
                        nc.tensor.transpose(out=xT_ps[:], in_=xb_t[:], identity=ident[:])
                        x0T = pool.tile([Cin, 128], F32, tag="x0T")
                        nc.vector.tensor_copy(x0T[:], xT_ps[:])
                        # t1T via transpose of t1_s block
                        t1b = pool.tile([128, Cin], F32, tag="t1b")
                        nc.sync.dma_start(out=t1b[:], in_=t1_s[li][bass.ds(i * 128, 128), :])
                        t1T_ps = psumt.tile([Cin, 128], F32, tag="t1Tps")
                        nc.tensor.transpose(out=t1T_ps[:], in_=t1b[:], identity=ident[:])
                        t1T = pool.tile([Cin, 128], F32, tag="t1T")
                        nc.vector.tensor_copy(t1T[:], t1T_ps[:])
                        # tx2T = 2*prop(t1) - x0
                        tx2T = pool.tile([Cin, 128], F32, tag="tx2T")
                        nc.vector.scalar_tensor_tensor(
                            out=tx2T[:], in0=s_ps[:], scalar=2.0, in1=x0T[:],
                            op0=mybir.AluOpType.mult, op1=mybir.AluOpType.subtract)
                        o_ps = psum.tile([128, Cout], F32, tag="ops")
                        nc.tensor.matmul(out=o_ps[:], lhsT=x0T[:], rhs=wks[li][0][:],
                                         start=True, stop=False)
                        nc.tensor.matmul(out=o_ps[:], lhsT=t1T[:], rhs=wks[li][1][:],
                                         start=False, stop=False)
                        nc.tensor.matmul(out=o_ps[:], lhsT=tx2T[:], rhs=wks[li][2][:],
                                         start=False, stop=True)
                        h_sb = pool.tile([128, Cout], F32, tag="hsb")
                        nc.vector.tensor_tensor(out=h_sb[:], in0=o_ps[:], in1=biases[li][:],
                                                op=mybir.AluOpType.add)
                        if relu:
                            nc.vector.tensor_scalar_max(out=h_sb[:], in0=h_sb[:], scalar1=0.0)
                        if li == len(dims) - 1:
                            nc.sync.dma_start(out=out_d[bass.ds(i * 128, 128), :], in_=h_sb[:])
                        else:
                            nc.sync.dma_start(out=h_s[li][bass.ds(i * 128, 128), :], in_=h_sb[:])

                    tc.For_i_unrolled(0, NB, 1, comb_body, max_unroll=unroll)

            for rep in range(reps):
                nc.sync.dma_start(out=xb[:], in_=xs_d[:])
                allgather(xb, xg)
                for li, (Cin, Cout, relu) in enumerate(dims):
                    v_prop = xg if li == 0 else h_g[li - 1]
                    x0_src = xs_d if li == 0 else h_s[li - 1]
                    emit_prop(rep, li, Cin, v_prop)
                    allgather(t1_s[li], t1_g[li])
                    emit_combine(rep, li, Cin, Cout, relu, x0_src)
                    if li < len(dims) - 1:
                        allgather(h_s[li], h_g[li])
    nc.finalize()
    return nc


# ------------------------------------------------------------- full model

class FusedModel:
    """Single fused program; constants staged once, warm runs re-stage x only."""

    def __init__(self, cfg, T, dims=((128, 64, True), (64, 128, True), (128, 256, False)),
                 unroll=2, **build_kw):
        self.cfg = cfg
        self.T = T
        self.dims = dims
        nc = build_fused(cfg, T, dims, unroll, **build_kw)
        self.runner = Runner(nc)
        self.dev_const = None
        self.dev_zero = None

    def stage_const(self, metas, weights):
        r = self.runner
        d = {}
        d["meta"] = r.put_one("meta", metas)
        d["iota"] = r.put_one("iota", iota_host())
        for li, (W, b) in enumerate(weights):
            Cin, Cout = self.dims[li][0], self.dims[li][1]
            wk = np.ascontiguousarray(np.asarray(W, np.float32).reshape(3 * Cin, Cout))
            bias_rep = np.broadcast_to(np.asarray(b, np.float32), (128, Cout)).copy()
            d[f"wk{li}"] = r.put_one(f"wk{li}", wk)
            d[f"bias{li}"] = r.put_one(f"bias{li}", bias_rep)
        self.dev_const = d
        self.dev_zero = r.zeros()

    def stage_x(self, x):
        """Per-device threaded put of each core's slot-layout x shard."""
        from concurrent.futures import ThreadPoolExecutor
        cfg = self.cfg
        C = x.shape[1]
        xs = np.zeros((NCORES, cfg.SLOTS, C), np.float32)
        xs[:, :cfg.NPC] = np.asarray(x, np.float32).reshape(NCORES, cfg.NPC, -1)
        devices = self.runner.mesh.devices.flatten()
        with ThreadPoolExecutor(NCORES) as ex:
            parts = list(ex.map(
                lambda c: jax.device_put(xs[c], devices[c]), range(NCORES)))
        return jax.make_array_from_single_device_arrays(
            (NCORES * cfg.SLOTS, C), self.runner.sharding, parts)

    def fetch(self, out_arr):
        """Threaded per-shard device->host copy (serial fetch is ~20x slower)."""
        from concurrent.futures import ThreadPoolExecutor
        cfg = self.cfg
        shards = sorted(out_arr.addressable_shards, key=lambda s: s.index[0].start or 0)
        with ThreadPoolExecutor(NCORES) as ex:
            parts = list(ex.map(lambda s: np.asarray(s.data), shards))
        h3 = np.stack(parts, axis=0)
        return np.ascontiguousarray(h3[:, :cfg.NPC]).reshape(cfg.N, -1)

    def run(self, x):
        r = self.runner
        dev_x = self.stage_x(x)
        args = [self.dev_const[n] if n != "xs" else dev_x for n in r.in_names]
        outs = r.fn(*args, *self.dev_zero)
        jax.block_until_ready(outs)
        return self.fetch(outs[0])


# ------------------------------------------------------------- numpy oracle

def numpy_reference(x, edge_index, weights):
    N = x.shape[0]
    src = np.asarray(edge_index[0], np.int64)
    dst = np.asarray(edge_index[1], np.int64)
    mask = src != dst
    deg = np.bincount(src[mask], minlength=N).astype(np.float32)
    dinv = np.where(deg > 0, (1.0 / np.sqrt(np.maximum(deg, 1.0))).astype(np.float32), 0.0).astype(np.float32)
    w = (-dinv[src] * dinv[dst] * mask).astype(np.float32)

    def prop(h):
        out = np.zeros_like(h)
        np.add.at(out, dst, w[:, None] * h[src])
        return out

    h = x.astype(np.float32)
    for li, (W, b) in enumerate(weights):
        tx0 = h
        tx1 = prop(h)
        tx2 = 2.0 * prop(tx1) - tx0
        h = tx0 @ W[0] + tx1 @ W[1] + tx2 @ W[2] + b
        if li < len(weights) - 1:
            h = np.maximum(h, 0.0)
    return h


# ------------------------------------------------------------------ entry

N_NODES = 100000
_cache = {}
_LAST = None


def kernel(x, edge_index, batch, W1, b1, W2, b2, W3, b3):
    global _LAST
    cfg = Cfg(n_nodes=N_NODES, npc=N_NODES // NCORES)
    weights = [
        (np.asarray(W1, np.float32), np.asarray(b1, np.float32)),
        (np.asarray(W2, np.float32), np.asarray(b2, np.float32)),
        (np.asarray(W3, np.float32), np.asarray(b3, np.float32)),
    ]
    metas, T = host_prep(cfg, edge_index)
    key = ("fused", T)
    if key not in _cache:
        _cache[key] = FusedModel(cfg, T)
    model = _cache[key]
    model.stage_const(metas, weights)
    out = model.run(np.asarray(x, np.float32))
    _LAST = model
    return out.astype(np.float32)
